# revision 1
# baseline (speedup 1.0000x reference)
"""CTRNN policy kernel for Trainium2 (8 NeuronCores, batch-parallel).

Reference computation (per batch element b, B=64, N=1024, OBS=64, A=16):
    I = E[b] @ obs[b]
    repeat 10x:  y = tanh(gain*(v+bias))*mask
                 v = (v + DT/tau * (-v + W[b]@y + I)) * mask
    action[b] = D[b] @ v

Sharding: batch 64 -> 8 cores x 8 individuals, fully data parallel.

Per-core algorithm (algebraic refactor to minimize per-iteration work):
    am = DT/tau*mask, cm = (1-DT/tau)*mask
    Wf = diag(am) @ W @ diag(mask)   (host-folded)
    Ef = diag(am) @ E                (host-folded)
    bc = bias*(1-cm)                 (host)
    state vs = v + bias; then per iteration:
        y   = tanh(g * vs)
        vs' = cm*vs + Wf@y + (Ef@obs + bc)
    finally action = D @ (vs - bias)

Layout: vector index n = p*8 + c maps to SBUF [p=partition(128), c=free(8)].
The matvec Wf@y runs on TensorE as 16 float32r matmuls per iteration:
stationary = y column chunk [128,1], moving = transposed-W slab [128,512],
accumulating into PSUM [1,1024] (row layout, n-ordered), which is then
fused-added with (Ef@obs+bc) on VectorE and DMA-scattered back to [128,8].
"""

import os
import sys
from contextlib import ExitStack

import numpy as np

for _p in ("/opt/trn_rl_repo", "/root/.axon_site/_ro/trn_rl_repo"):
    if os.path.isdir(_p) and _p not in sys.path:
        sys.path.append(_p)

import concourse.bass as bass  # noqa: E402
import concourse.tile as tile  # noqa: E402
from concourse import bacc, mybir  # noqa: E402
from concourse.bass_utils import run_bass_kernel_spmd  # noqa: E402

DT = 0.1
ITERS = int(1.0 // DT)  # == 9: reference.py uses `int(1.0 // DT)`, and 1.0//0.1 == 9.0
B_FULL, N, OBS, ADIM = 64, 1024, 64, 16
NCORES = 8
BPC = B_FULL // NCORES  # individuals per core
P, CN = 128, 8          # n = p*8 + c
F32 = mybir.dt.float32
F32R = mybir.dt.float32r
GROUPS = [(0, 1, 2), (3, 4, 5), (6, 7)]


def make_pools(ctx, tc):
    return dict(
        const=ctx.enter_context(tc.tile_pool(name="const", bufs=1)),
        wpool=ctx.enter_context(tc.tile_pool(name="w", bufs=4)),
        etpool=ctx.enter_context(tc.tile_pool(name="et", bufs=2)),
        rowpool=ctx.enter_context(tc.tile_pool(name="row", bufs=3)),
        scat=ctx.enter_context(tc.tile_pool(name="scat", bufs=6)),
        tmp=ctx.enter_context(tc.tile_pool(name="tmp", bufs=6)),
        pspool=ctx.enter_context(tc.tile_pool(name="ps", bufs=3, space="PSUM")),
        psact=ctx.enter_context(tc.tile_pool(name="psa", bufs=1, space="PSUM")),
    )


def kernel_body(ctx, tc, ins, out_ap, iters=ITERS, pools=None, probe_no_chain=False):
    nc = tc.nc
    Tanh = mybir.ActivationFunctionType.Tanh
    add = mybir.AluOpType.add
    mult = mybir.AluOpType.mult
    sub = mybir.AluOpType.subtract

    p = pools if pools is not None else make_pools(ctx, tc)
    const, wpool, etpool, rowpool = p["const"], p["wpool"], p["etpool"], p["rowpool"]
    scat, tmp, pspool, psact = p["scat"], p["tmp"], p["pspool"], p["psact"]

    # ---- constants / persistent state ----
    obs_sb = const.tile([OBS, BPC], F32, tag="obs", name="obs")
    nc.sync.dma_start(obs_sb[:], ins["obsT"][:])
    # per-individual [1, N] row tiles at partition 0 (engine ops need aligned
    # start partitions); seeded with bc, then += Ef@obs on device
    ifb_sb = {}
    for b in range(BPC):
        ifb_sb[b] = const.tile([1, N], F32, tag=f"ifb{b}", name=f"ifb{b}")
        nc.sync.dma_start(ifb_sb[b][:], ins["bc"][b])
    dtr_sb = const.tile([P, BPC * CN * ADIM], F32, tag="dtr", name="dtr")  # [128, 1024]
    for b in range(BPC):
        nc.sync.dma_start(dtr_sb[:, b * CN * ADIM:(b + 1) * CN * ADIM], ins["DTr"][b])

    cm_sb, g_sb, bias_sb, vs_sb, y_sb = {}, {}, {}, {}, {}
    for b in range(BPC):
        cm_sb[b] = const.tile([P, CN], F32, tag=f"cm{b}", name=f"cm{b}")
        nc.sync.dma_start(cm_sb[b][:], ins["cm"][b])
        g_sb[b] = const.tile([P, CN], F32, tag=f"g{b}", name=f"g{b}")
        nc.sync.dma_start(g_sb[b][:], ins["g"][b])
        bias_sb[b] = const.tile([P, CN], F32, tag=f"bias{b}", name=f"bias{b}")
        nc.sync.dma_start(bias_sb[b][:], ins["biasS"][b])
        vs_sb[b] = const.tile([P, CN], F32, tag=f"vs{b}", name=f"vs{b}")
        nc.sync.dma_start(vs_sb[b][:], ins["vs0"][b])
        y_sb[b] = const.tile([P, CN], F32R, tag=f"y{b}", name=f"y{b}")

    act_sb = const.tile([1, BPC * ADIM], F32, tag="act", name="act")

    # ---- W loads (slot-limited by pool bufs; scheduler orders them) ----
    w_sb = {}
    for b in range(BPC):
        w_sb[b] = wpool.tile([P, CN * N], F32R, tag="w", name="w")
        nc.sync.dma_start(w_sb[b][:], ins["Wf"][b])

    # ---- per-individual setup: input current + initial y ----
    for b in range(BPC):
        et = etpool.tile([OBS, N], F32, tag="et", name="et")
        nc.sync.dma_start(et[:], ins["ET"][b])
        ip = pspool.tile([1, N], F32, tag="ps", name="ps")
        for h in range(2):
            nc.tensor.matmul(
                ip[0:1, h * 512:(h + 1) * 512],
                obs_sb[:, b:b + 1],
                et[:, h * 512:(h + 1) * 512],
                start=True, stop=True,
            )
        # Ifb[b] = (Ef@obs) + bc[b]   (in-place: tile was seeded with bc)
        nc.vector.tensor_tensor(ifb_sb[b][:], ip[0:1, :], ifb_sb[b][:], op=add)
        # y0 = tanh(g * vs0)
        t2 = tmp.tile([P, CN], F32, tag="t2", name="t2")
        nc.vector.tensor_tensor(t2[:], g_sb[b][:], vs_sb[b][:], op=mult)
        nc.scalar.activation(y_sb[b][:], t2[:], Tanh)

    # ---- recurrent loop: groups of individuals interleaved per iteration ----
    for group in GROUPS:
        for t in range(iters):
            for b in group:
                wy = pspool.tile([1, N], F32, tag="ps", name="ps")
                for c in range(CN):
                    yc = y_sb[b][:, c:c + 1]
                    for h in range(2):
                        nc.tensor.matmul(
                            wy[0:1, h * 512:(h + 1) * 512],
                            yc,
                            w_sb[b][:, c * N + h * 512: c * N + h * 512 + 512],
                            start=(c == 0), stop=(c == CN - 1),
                        )
                if probe_no_chain:
                    continue
                u_row = rowpool.tile([1, N], F32, tag="urow", name="urow")
                nc.vector.tensor_tensor(u_row[:], wy[0:1, :], ifb_sb[b][:], op=add)
                u = scat.tile([P, CN], F32, tag="u", name="u")
                nc.sync.dma_start(u[:], u_row[:])  # [1,1024] -> [128,8], n = p*8+c
                t1 = tmp.tile([P, CN], F32, tag="t1", name="t1")
                nc.vector.tensor_tensor(t1[:], cm_sb[b][:], vs_sb[b][:], op=mult)
                nc.vector.tensor_tensor(vs_sb[b][:], t1[:], u[:], op=add)
                if t < iters - 1:
                    t2 = tmp.tile([P, CN], F32, tag="t2", name="t2")
                    nc.vector.tensor_tensor(t2[:], g_sb[b][:], vs_sb[b][:], op=mult)
                    nc.scalar.activation(y_sb[b][:], t2[:], Tanh)

    # ---- decode: action = D @ (vs - bias) ----
    for b in range(BPC):
        vf = tmp.tile([P, CN], F32, tag="vf", name="vf")
        nc.vector.tensor_tensor(vf[:], vs_sb[b][:], bias_sb[b][:], op=sub)
        ap = psact.tile([1, ADIM], F32, tag="psa", name="psa")
        for c in range(CN):
            nc.tensor.matmul(
                ap[0:1, :],
                vf[:, c:c + 1],
                dtr_sb[:, b * CN * ADIM + c * ADIM: b * CN * ADIM + (c + 1) * ADIM],
                start=(c == 0), stop=(c == CN - 1),
            )
        nc.vector.tensor_copy(act_sb[0:1, b * ADIM:(b + 1) * ADIM], ap[0:1, :])
    nc.sync.dma_start(out_ap[:], act_sb[0:1, :])


def build_nc(iters=ITERS, reps=1, probe_no_chain=False):
    nc = bacc.Bacc(
        "TRN2", target_bir_lowering=False, debug=False, enable_asserts=False,
    )
    ins = {}
    for name, shape in [
        ("ET", [BPC, OBS, N]),
        ("DTr", [BPC, P, CN * ADIM]),
        ("obsT", [OBS, BPC]),
        ("vs0", [BPC, P, CN]),
        ("cm", [BPC, P, CN]),
        ("g", [BPC, P, CN]),
        ("biasS", [BPC, P, CN]),
        ("bc", [BPC, N]),
    ]:
        ins[name] = nc.dram_tensor(name, shape, F32, kind="ExternalInput").ap()
    ins["Wf"] = nc.dram_tensor("Wf", [BPC, P, CN * N], F32R, kind="ExternalInput").ap()
    out_ap = nc.dram_tensor("act", [BPC, ADIM], F32, kind="ExternalOutput").ap()

    with tile.TileContext(nc) as tc:
        with ExitStack() as ctx:
            pools = make_pools(ctx, tc)
            for _rep in range(reps):
                kernel_body(ctx, tc, ins, out_ap, iters, pools, probe_no_chain)
    nc.compile()
    return nc


def _round_tf32(x):
    """Round fp32 array to tf32 (10-bit mantissa), round-to-nearest-even."""
    u = x.view(np.uint32)
    u = u + (0x0FFF + ((u >> 13) & 1))
    u &= np.uint32(0xFFFFE000)
    return u.view(np.float32)


def prep_in_maps(obs, v0, tau, gain, bias, W, mask, E, D):
    f = np.float32
    obs, v0, tau, gain, bias, W, mask, E, D = [
        np.asarray(x, dtype=f) for x in (obs, v0, tau, gain, bias, W, mask, E, D)
    ]
    am = (DT / tau) * mask                    # [64, N]
    cm = (1.0 - DT / tau) * mask
    Wf = W * am[:, :, None] * mask[:, None, :]
    WT = np.ascontiguousarray(Wf.transpose(0, 2, 1)).reshape(B_FULL, P, CN * N)
    WT = _round_tf32(WT)
    ETp = np.ascontiguousarray((E * am[:, :, None]).transpose(0, 2, 1))  # [64, OBS, N]
    DTp = np.ascontiguousarray(D.transpose(0, 2, 1)).reshape(B_FULL, P, CN * ADIM)
    obsT = np.ascontiguousarray(obs.T)        # [OBS, 64]
    vs0 = np.ascontiguousarray((v0 + bias).reshape(B_FULL, P, CN))
    cmS = np.ascontiguousarray(cm.reshape(B_FULL, P, CN))
    gS = np.ascontiguousarray(gain.reshape(B_FULL, P, CN))
    bS = np.ascontiguousarray(bias.reshape(B_FULL, P, CN))
    bc = np.ascontiguousarray(bias * (1.0 - cm))  # [64, N]

    in_maps = []
    for core in range(NCORES):
        s = slice(core * BPC, (core + 1) * BPC)
        in_maps.append({
            "Wf": np.ascontiguousarray(WT[s]),
            "ET": np.ascontiguousarray(ETp[s]),
            "DTr": np.ascontiguousarray(DTp[s]),
            "obsT": np.ascontiguousarray(obsT[:, s]),
            "vs0": vs0[s], "cm": cmS[s], "g": gS[s], "biasS": bS[s],
            "bc": bc[s],
        })
    return in_maps


_NC_CACHE = None


def _get_nc():
    global _NC_CACHE
    if _NC_CACHE is None:
        _NC_CACHE = build_nc()
    return _NC_CACHE


def kernel(obs, v0, tau, gain, bias, W, mask, E, D):
    nc = _get_nc()
    in_maps = prep_in_maps(obs, v0, tau, gain, bias, W, mask, E, D)
    res = run_bass_kernel_spmd(nc, in_maps, core_ids=list(range(NCORES)))
    return np.concatenate([res.results[c]["act"] for c in range(NCORES)], axis=0)



# revision 3
# speedup vs baseline: 1.2768x; 1.2768x over previous
"""CTRNN policy kernel for Trainium2 (8 NeuronCores, batch-parallel).

Reference computation (per batch element b, B=64, N=1024, OBS=64, A=16):
    I = E[b] @ obs[b]
    repeat int(1.0//0.1)=9 times:
        y = tanh(gain*(v+bias))*mask
        v = (v + DT/tau * (-v + W[b]@y + I)) * mask
    action[b] = D[b] @ v

Sharding: batch 64 -> 8 cores x 8 individuals, fully data parallel.

Per-core algebra (host-folded, mask/tau folded into the coefficients):
    am = DT/tau*mask, cm = (1-DT/tau)*mask
    Wf = diag(am) @ W @ diag(mask);  Ef = diag(am) @ E;  bc = bias*(1-cm)
    state vs = v + bias:
        y   = tanh(g * vs)
        vs' = cm*vs + Wf@y + (Ef@obs + bc)
    action = D @ (vs - bias)

Device mapping per individual (N=1024 split as n = p*8 + c, p=partition):
  - W^T slabs [128, 8192] bf16 resident in SBUF for all 8 individuals.
  - matvec on TensorE: stationary = y column chunk [128,1] (bf16), moving =
    W^T slab. COLGROUPS=1: 16 x N=512 matmuls into a PSUM row [1,1024].
    COLGROUPS=2/4: column-group tiling (tile_position=(0,32j)) runs 2/4
    concurrent streams on separate PSUM banks/partitions.
  - evacuate PSUM row(s) to SBUF (scalar/vector copies), DMA-scatter
    [rows] -> [128,8] column layout (ACT-ring HWDGE, separate FIFO from the
    bulk W loads on the SP ring), then cheap [128,8] vector ops + tanh.
"""

import os
import sys
from contextlib import ExitStack

import numpy as np

for _p in ("/opt/trn_rl_repo", "/root/.axon_site/_ro/trn_rl_repo"):
    if os.path.isdir(_p) and _p not in sys.path:
        sys.path.append(_p)

import ml_dtypes  # noqa: E402

import concourse.bass as bass  # noqa: E402
import concourse.tile as tile  # noqa: E402
from concourse import bacc, mybir  # noqa: E402
from concourse.bass_utils import run_bass_kernel_spmd  # noqa: E402

DT = 0.1
ITERS = int(1.0 // DT)  # == 9: 1.0//0.1 == 9.0 in fp
B_FULL, N, OBS, ADIM = 64, 1024, 64, 16
NCORES = 8
BPC = B_FULL // NCORES
P, CN = 128, 8          # n = p*8 + c
F32 = mybir.dt.float32
BF16 = mybir.dt.bfloat16
BF16_NP = ml_dtypes.bfloat16

COLGROUPS = int(os.environ.get("CTRNN_COLGROUPS", "1"))
assert COLGROUPS in (1, 2, 4)
NSLAB = N // COLGROUPS
# matmul sub-slabs: (tile_col, psum_off, w_off, width) per column group
if COLGROUPS == 1:
    GSLABS = [(0, 0, 0, 512), (0, 512, 512, 512)]
    PSW = 1024
elif COLGROUPS == 2:
    GSLABS = [(0, 0, 0, 512), (32, 512, 512, 512)]
    PSW = 1024
else:
    GSLABS = [(32 * j, 512 * j, 256 * j, 256) for j in range(4)]
    PSW = 2048
PS_BUFS = 2 if COLGROUPS == 4 else 3
# phases: first 3 individuals run while W 3..7 stream in
PHASES = [(0, 1, 2), (3, 4, 5, 6, 7)]
# consts packing: per individual 5 blocks of 8 cols: cm, g, bias, vs0, bc
NCONST = 5


def cidx(b, k):
    return (b * NCONST + k) * CN


def make_pools(ctx, tc):
    return dict(
        const=ctx.enter_context(tc.tile_pool(name="const", bufs=1)),
        wpool=ctx.enter_context(tc.tile_pool(name="w", bufs=BPC)),
        row=ctx.enter_context(tc.tile_pool(name="row", bufs=3)),
        ucol=ctx.enter_context(tc.tile_pool(name="ucol", bufs=6)),
        tmp=ctx.enter_context(tc.tile_pool(name="tmp", bufs=8)),
        ps=ctx.enter_context(tc.tile_pool(name="ps", bufs=PS_BUFS, space="PSUM")),
    )


def kernel_body(ctx, tc, ins, out_ap, iters=ITERS, pools=None):
    nc = tc.nc
    Tanh = mybir.ActivationFunctionType.Tanh
    add = mybir.AluOpType.add
    mult = mybir.AluOpType.mult
    sub = mybir.AluOpType.subtract

    p = pools if pools is not None else make_pools(ctx, tc)
    const, wpool, row, ucol, tmp, ps = (
        p["const"], p["wpool"], p["row"], p["ucol"], p["tmp"], p["ps"])

    # ---- bulk loads on the SP (sync) HWDGE ring: W first ----
    w_sb = []
    for b in range(BPC):
        w_sb.append(wpool.tile([P, CN * N], BF16, tag="w", name=f"w{b}"))
    nc.sync.dma_start(w_sb[0][:], ins["WT"][0])

    consts = const.tile([P, BPC * NCONST * CN], F32, tag="consts", name="consts")
    nc.sync.dma_start(consts[:], ins["consts"][:])
    obs_sb = const.tile([OBS, BPC], BF16, tag="obs", name="obs")
    nc.sync.dma_start(obs_sb[:], ins["obsT"][:])
    dt_sb = const.tile([P, BPC * CN * ADIM], F32, tag="dt", name="dt")
    nc.sync.dma_start(dt_sb[:], ins["DTall"][:])
    et_sb = const.tile([OBS, BPC * N], BF16, tag="et", name="et")
    nc.sync.dma_start(et_sb[:], ins["ETall"][:])

    for b in range(1, BPC):
        nc.sync.dma_start(w_sb[b][:], ins["WT"][b])

    # ---- per-individual state ----
    vs_sb, y_sb, icol_sb = [], [], []
    for b in range(BPC):
        vs_sb.append(const.tile([P, CN], F32, tag=f"vs{b}", name=f"vs{b}"))
        y_sb.append(const.tile([P, CN], BF16, tag=f"y{b}", name=f"y{b}"))
        icol_sb.append(const.tile([P, CN], F32, tag=f"ic{b}", name=f"ic{b}"))
    act_sb = const.tile([1, BPC * ADIM], F32, tag="act", name="act")

    def cm_ap(b):
        return consts[:, cidx(b, 0):cidx(b, 0) + CN]

    def g_ap(b):
        return consts[:, cidx(b, 1):cidx(b, 1) + CN]

    def bias_ap(b):
        return consts[:, cidx(b, 2):cidx(b, 2) + CN]

    def vs0_ap(b):
        return consts[:, cidx(b, 3):cidx(b, 3) + CN]

    def bc_ap(b):
        return consts[:, cidx(b, 4):cidx(b, 4) + CN]

    # ---- setup: I = Ef@obs (+bc) into column layout; y0 = tanh(g*vs0) ----
    for b in range(BPC):
        ip = ps.tile([P, PSW], F32, tag="ps", name=f"ip{b}")
        for h in range(2):
            nc.tensor.matmul(
                ip[0:1, h * 512:(h + 1) * 512],
                obs_sb[:, b:b + 1],
                et_sb[:, b * N + h * 512: b * N + h * 512 + 512],
                start=True, stop=True,
            )
        ir = row.tile([1, N], F32, tag="irow", name=f"ir{b}")
        nc.scalar.copy(ir[:], ip[0:1, 0:N])
        itmp = ucol.tile([P, CN], F32, tag="ucol", name=f"it{b}")
        nc.scalar.dma_start(itmp[:], ir[:])  # [1,1024] -> [128,8]
        nc.vector.tensor_tensor(icol_sb[b][:], itmp[:], bc_ap(b), op=add)
        # vs = vs0; y0 = tanh(g*vs0)
        nc.vector.tensor_copy(vs_sb[b][:], vs0_ap(b))
        s0 = tmp.tile([P, CN], F32, tag="s", name=f"s0{b}")
        nc.vector.tensor_tensor(s0[:], g_ap(b), vs0_ap(b), op=mult)
        nc.scalar.activation(y_sb[b][:], s0[:], Tanh)

    # ---- recurrent loop ----
    def matvec(b, t):
        wy = ps.tile([P, PSW], F32, tag="ps", name="wy")
        for c in range(CN):
            yc = y_sb[b][:, c:c + 1]
            for (tcol, poff, woff, width) in GSLABS:
                nc.tensor.matmul(
                    wy[tcol:tcol + 1, poff:poff + width],
                    yc,
                    w_sb[b][:, c * N + woff: c * N + woff + width],
                    start=(c == 0), stop=(c == CN - 1),
                    tile_position=(0, tcol) if COLGROUPS > 1 else None,
                )
        u4 = row.tile([P, NSLAB] if COLGROUPS > 1 else [1, N], F32,
                      tag="u4", name="u4")
        for i, (tcol, poff, woff, width) in enumerate(GSLABS):
            src = wy[tcol:tcol + 1, poff:poff + width]
            if COLGROUPS == 1:
                dst = u4[0:1, woff:woff + width]
            else:
                dst = u4[tcol:tcol + 1, 0:width]
            if i % 2 == 0:
                nc.scalar.copy(dst, src)
            else:
                nc.vector.tensor_copy(dst, src)
        ucl = ucol.tile([P, CN], F32, tag="ucol", name="u")
        if COLGROUPS == 1:
            scat_src = u4[0:1, :]
        else:
            scat_src = u4[0:32 * COLGROUPS:32, :]
        nc.scalar.dma_start(ucl[:], scat_src)
        t0 = tmp.tile([P, CN], F32, tag="t0", name="t0")
        nc.vector.tensor_tensor(t0[:], ucl[:], icol_sb[b][:], op=add)
        t1 = tmp.tile([P, CN], F32, tag="t1", name="t1")
        nc.vector.tensor_tensor(t1[:], cm_ap(b), vs_sb[b][:], op=mult)
        nc.vector.tensor_tensor(vs_sb[b][:], t0[:], t1[:], op=add)
        if t < iters - 1:
            s = tmp.tile([P, CN], F32, tag="s", name="s")
            nc.vector.tensor_tensor(s[:], g_ap(b), vs_sb[b][:], op=mult)
            nc.scalar.activation(y_sb[b][:], s[:], Tanh)

    def decode(b):
        vf = tmp.tile([P, CN], F32, tag="vf", name="vf")
        nc.vector.tensor_tensor(vf[:], vs_sb[b][:], bias_ap(b), op=sub)
        ap = ps.tile([P, PSW], F32, tag="ps", name="dec")
        for c in range(CN):
            nc.tensor.matmul(
                ap[0:1, 0:ADIM],
                vf[:, c:c + 1],
                dt_sb[:, b * CN * ADIM + c * ADIM: b * CN * ADIM + (c + 1) * ADIM],
                start=(c == 0), stop=(c == CN - 1),
            )
        nc.vector.tensor_copy(act_sb[0:1, b * ADIM:(b + 1) * ADIM], ap[0:1, 0:ADIM])

    for phase in PHASES:
        for t in range(iters):
            for b in phase:
                matvec(b, t)
        for b in phase:
            decode(b)
    nc.sync.dma_start(out_ap[:], act_sb[0:1, :])


def build_nc(iters=ITERS):
    nc = bacc.Bacc(
        "TRN2", target_bir_lowering=False, debug=False, enable_asserts=False,
    )
    ins = {}
    ins["WT"] = nc.dram_tensor("WT", [BPC, P, CN * N], BF16, kind="ExternalInput").ap()
    ins["ETall"] = nc.dram_tensor("ETall", [OBS, BPC * N], BF16, kind="ExternalInput").ap()
    ins["obsT"] = nc.dram_tensor("obsT", [OBS, BPC], BF16, kind="ExternalInput").ap()
    ins["consts"] = nc.dram_tensor(
        "consts", [P, BPC * NCONST * CN], F32, kind="ExternalInput").ap()
    ins["DTall"] = nc.dram_tensor(
        "DTall", [P, BPC * CN * ADIM], F32, kind="ExternalInput").ap()
    out_ap = nc.dram_tensor("act", [BPC, ADIM], F32, kind="ExternalOutput").ap()

    with tile.TileContext(nc) as tc:
        with ExitStack() as ctx:
            pools = make_pools(ctx, tc)
            kernel_body(ctx, tc, ins, out_ap, iters, pools)
    nc.compile()
    return nc


def prep_in_maps(obs, v0, tau, gain, bias, W, mask, E, D):
    f = np.float32
    obs, v0, tau, gain, bias, W, mask, E, D = [
        np.asarray(x, dtype=f) for x in (obs, v0, tau, gain, bias, W, mask, E, D)
    ]
    am = (DT / tau) * mask                    # [64, N]
    cm = (1.0 - DT / tau) * mask
    Wf = W * am[:, :, None] * mask[:, None, :]
    WT = np.ascontiguousarray(Wf.transpose(0, 2, 1)).reshape(
        B_FULL, P, CN * N).astype(BF16_NP)
    ETp = np.ascontiguousarray(
        (E * am[:, :, None]).transpose(0, 2, 1)).astype(BF16_NP)  # [64, OBS, N]
    DTp = np.ascontiguousarray(D.transpose(0, 2, 1)).reshape(B_FULL, P, CN * ADIM)
    obsT = np.ascontiguousarray(obs.T).astype(BF16_NP)  # [OBS, 64]
    vs0 = (v0 + bias).reshape(B_FULL, P, CN)
    cmS = cm.reshape(B_FULL, P, CN)
    gS = gain.reshape(B_FULL, P, CN)
    bS = bias.reshape(B_FULL, P, CN)
    bcS = (bias * (1.0 - cm)).reshape(B_FULL, P, CN)

    in_maps = []
    for core in range(NCORES):
        s = slice(core * BPC, (core + 1) * BPC)
        # consts [128, BPC*5*8]: per b: cm, g, bias, vs0, bc
        cst = np.empty((P, BPC * NCONST * CN), f)
        for i, b in enumerate(range(core * BPC, (core + 1) * BPC)):
            for k, arr in enumerate((cmS, gS, bS, vs0, bcS)):
                cst[:, (i * NCONST + k) * CN:(i * NCONST + k + 1) * CN] = arr[b]
        et = np.ascontiguousarray(
            ETp[s].transpose(1, 0, 2).reshape(OBS, BPC * N))
        dtall = np.ascontiguousarray(
            DTp[s].transpose(1, 0, 2).reshape(P, BPC * CN * ADIM))
        in_maps.append({
            "WT": np.ascontiguousarray(WT[s]),
            "ETall": et,
            "obsT": np.ascontiguousarray(obsT[:, s]),
            "consts": cst,
            "DTall": dtall,
        })
    return in_maps


_NC_CACHE = None


def _get_nc():
    global _NC_CACHE
    if _NC_CACHE is None:
        _NC_CACHE = build_nc()
    return _NC_CACHE


def kernel(obs, v0, tau, gain, bias, W, mask, E, D):
    nc = _get_nc()
    in_maps = prep_in_maps(obs, v0, tau, gain, bias, W, mask, E, D)
    res = run_bass_kernel_spmd(nc, in_maps, core_ids=list(range(NCORES)))
    return np.concatenate([res.results[c]["act"] for c in range(NCORES)], axis=0)


# revision 6
# speedup vs baseline: 1.4558x; 1.1402x over previous
"""CTRNN policy kernel for Trainium2 (8 NeuronCores, batch-parallel).

Reference computation (per batch element b, B=64, N=1024, OBS=64, A=16):
    I = E[b] @ obs[b]
    repeat int(1.0//0.1)=9 times:
        y = tanh(gain*(v+bias))*mask
        v = (v + DT/tau * (-v + W[b]@y + I)) * mask
    action[b] = D[b] @ v

Sharding: batch 64 -> 8 cores x 8 individuals, fully data parallel.

Per-core algebra (host-folded, mask/tau folded into the coefficients):
    am = DT/tau*mask, cm = (1-DT/tau)*mask
    Wf = diag(am) @ W @ diag(mask);  Ef = diag(am) @ E;  bc = bias*(1-cm)
    state vs = v + bias:
        y   = tanh(g * vs)
        vs' = cm*vs + Wf@y + (Ef@obs + bc)
    action = D @ (vs - bias)

Device mapping per individual (N=1024 split as n = p*8 + c, p=partition):
  - W^T slabs [128, 8192] bf16 resident in SBUF for all 8 individuals.
  - matvec on TensorE: stationary = y column chunk [128,1] (bf16), moving =
    W^T slab. COLGROUPS=1: 16 x N=512 matmuls into a PSUM row [1,1024].
    COLGROUPS=2/4: column-group tiling (tile_position=(0,32j)) runs 2/4
    concurrent streams on separate PSUM banks/partitions.
  - evacuate PSUM row(s) to SBUF (scalar/vector copies), DMA-scatter
    [rows] -> [128,8] column layout (ACT-ring HWDGE, separate FIFO from the
    bulk W loads on the SP ring), then cheap [128,8] vector ops + tanh.
"""

import os
import sys
from contextlib import ExitStack

import numpy as np

for _p in ("/opt/trn_rl_repo", "/root/.axon_site/_ro/trn_rl_repo"):
    if os.path.isdir(_p) and _p not in sys.path:
        sys.path.append(_p)

import ml_dtypes  # noqa: E402

import concourse.bass as bass  # noqa: E402
import concourse.tile as tile  # noqa: E402
from concourse import bacc, mybir  # noqa: E402
from concourse.bass_utils import run_bass_kernel_spmd  # noqa: E402

DT = 0.1
ITERS = int(1.0 // DT)  # == 9: 1.0//0.1 == 9.0 in fp
B_FULL, N, OBS, ADIM = 64, 1024, 64, 16
NCORES = 8
BPC = B_FULL // NCORES
P, CN = 128, 8          # n = p*8 + c
F32 = mybir.dt.float32
BF16 = mybir.dt.bfloat16
BF16_NP = ml_dtypes.bfloat16

COLGROUPS = int(os.environ.get("CTRNN_COLGROUPS", "1"))
assert COLGROUPS in (1, 2, 4)
NSLAB = N // COLGROUPS
# matmul sub-slabs: (tile_col, psum_off, w_off, width) per column group
if COLGROUPS == 1:
    GSLABS = [(0, 0, 0, 512), (0, 512, 512, 512)]
    PSW = 1024
    PS_BUFS = 3
elif COLGROUPS == 2:
    GSLABS = [(0, 0, 0, 512), (32, 512, 512, 512)]
    PSW = 1024
    PS_BUFS = 3
else:
    # all 4 column groups share one PSUM bank: disjoint partitions {0,32,64,96},
    # same free range [0:256) -> single-bank tiles, deep pipeline, 1-copy evac
    GSLABS = [(32 * j, 0, 256 * j, 256) for j in range(4)]
    PSW = 512
    PS_BUFS = 8
# phases: first 3 individuals run while W 3..7 stream in
PHASES = [(0, 1, 2), (3, 4, 5, 6, 7)]
# consts packing: per individual 5 blocks of 8 cols: cm, g, bias, vs0, bc
NCONST = 5


def cidx(b, k):
    return (b * NCONST + k) * CN


def make_pools(ctx, tc):
    return dict(
        const=ctx.enter_context(tc.tile_pool(name="const", bufs=1)),
        wpool=ctx.enter_context(tc.tile_pool(name="w", bufs=BPC)),
        row=ctx.enter_context(tc.tile_pool(name="row", bufs=3)),
        ucol=ctx.enter_context(tc.tile_pool(name="ucol", bufs=6)),
        tmp=ctx.enter_context(tc.tile_pool(name="tmp", bufs=8)),
        ps=ctx.enter_context(tc.tile_pool(name="ps", bufs=PS_BUFS, space="PSUM")),
    )


def kernel_body(ctx, tc, ins, out_ap, iters=ITERS, pools=None):
    nc = tc.nc
    Tanh = mybir.ActivationFunctionType.Tanh
    add = mybir.AluOpType.add
    mult = mybir.AluOpType.mult
    sub = mybir.AluOpType.subtract

    p = pools if pools is not None else make_pools(ctx, tc)
    const, wpool, row, ucol, tmp, ps = (
        p["const"], p["wpool"], p["row"], p["ucol"], p["tmp"], p["ps"])

    # ---- bulk loads on the SP (sync) HWDGE ring: W first ----
    w_sb = []
    for b in range(BPC):
        w_sb.append(wpool.tile([P, CN * N], BF16, tag="w", name=f"w{b}"))
    nc.sync.dma_start(w_sb[0][:], ins["WT"][0])

    consts = const.tile([P, BPC * NCONST * CN], F32, tag="consts", name="consts")
    nc.sync.dma_start(consts[:], ins["consts"][:])
    obs_sb = const.tile([OBS, BPC], BF16, tag="obs", name="obs")
    nc.sync.dma_start(obs_sb[:], ins["obsT"][:])
    dt_sb = const.tile([P, BPC * CN * ADIM], F32, tag="dt", name="dt")
    nc.sync.dma_start(dt_sb[:], ins["DTall"][:])
    et_sb = const.tile([OBS, BPC * N], BF16, tag="et", name="et")
    nc.sync.dma_start(et_sb[:], ins["ETall"][:])

    for b in range(1, BPC):
        nc.sync.dma_start(w_sb[b][:], ins["WT"][b])

    # ---- per-individual state ----
    vs_sb, y_sb, icol_sb = [], [], []
    for b in range(BPC):
        vs_sb.append(const.tile([P, CN], F32, tag=f"vs{b}", name=f"vs{b}"))
        y_sb.append(const.tile([P, CN], BF16, tag=f"y{b}", name=f"y{b}"))
        icol_sb.append(const.tile([P, CN], F32, tag=f"ic{b}", name=f"ic{b}"))
    act_sb = const.tile([1, BPC * ADIM], F32, tag="act", name="act")

    def cm_ap(b):
        return consts[:, cidx(b, 0):cidx(b, 0) + CN]

    def g_ap(b):
        return consts[:, cidx(b, 1):cidx(b, 1) + CN]

    def bias_ap(b):
        return consts[:, cidx(b, 2):cidx(b, 2) + CN]

    def vs0_ap(b):
        return consts[:, cidx(b, 3):cidx(b, 3) + CN]

    def bc_ap(b):
        return consts[:, cidx(b, 4):cidx(b, 4) + CN]

    # ---- setup: I = Ef@obs (+bc) into column layout; y0 = tanh(g*vs0) ----
    for b in range(BPC):
        ir = row.tile([1, N], F32, tag="irow", name=f"ir{b}")
        for h in range(2):
            ip = ps.tile([P, PSW], F32, tag="ps", name=f"ip{b}_{h}")
            nc.tensor.matmul(
                ip[0:1, 0:512],
                obs_sb[:, b:b + 1],
                et_sb[:, b * N + h * 512: b * N + h * 512 + 512],
                start=True, stop=True,
            )
            nc.scalar.copy(ir[0:1, h * 512:(h + 1) * 512], ip[0:1, 0:512])
        itmp = ucol.tile([P, CN], F32, tag="ucol", name=f"it{b}")
        nc.scalar.dma_start(itmp[:], ir[:])  # [1,1024] -> [128,8]
        nc.vector.tensor_tensor(icol_sb[b][:], itmp[:], bc_ap(b), op=add)
        # vs = vs0; y0 = tanh(g*vs0)
        nc.vector.tensor_copy(vs_sb[b][:], vs0_ap(b))
        s0 = tmp.tile([P, CN], F32, tag="s", name=f"s0{b}")
        nc.vector.tensor_tensor(s0[:], g_ap(b), vs0_ap(b), op=mult)
        nc.scalar.activation(y_sb[b][:], s0[:], Tanh)

    # ---- recurrent loop ----
    def matvec(b, t):
        wy = ps.tile([P, PSW], F32, tag="ps", name="wy")
        for c in range(CN):
            yc = y_sb[b][:, c:c + 1]
            for (tcol, poff, woff, width) in GSLABS:
                nc.tensor.matmul(
                    wy[tcol:tcol + 1, poff:poff + width],
                    yc,
                    w_sb[b][:, c * N + woff: c * N + woff + width],
                    start=(c == 0), stop=(c == CN - 1),
                    tile_position=(0, tcol) if COLGROUPS > 1 else None,
                )
        u4 = row.tile([P, NSLAB] if COLGROUPS > 1 else [1, N], F32,
                      tag="u4", name="u4")
        if COLGROUPS == 4:
            # all groups live in one bank at partitions {0,32,64,96}; one
            # 128-lane copy evacuates them all (garbage lanes are unused)
            nc.vector.tensor_copy(u4[:, :], wy[:, 0:NSLAB])
        else:
            for i, (tcol, poff, woff, width) in enumerate(GSLABS):
                src = wy[tcol:tcol + 1, poff:poff + width]
                if COLGROUPS == 1:
                    dst = u4[0:1, woff:woff + width]
                else:
                    dst = u4[tcol:tcol + 1, 0:width]
                if i % 2 == 0:
                    nc.scalar.copy(dst, src)
                else:
                    nc.vector.tensor_copy(dst, src)
        ucl = ucol.tile([P, CN], F32, tag="ucol", name="u")
        if COLGROUPS == 1:
            scat_src = u4[0:1, :]
        else:
            scat_src = u4[0:32 * COLGROUPS:32, :]
        nc.scalar.dma_start(ucl[:], scat_src)
        t0 = tmp.tile([P, CN], F32, tag="t0", name="t0")
        nc.vector.tensor_tensor(t0[:], ucl[:], icol_sb[b][:], op=add)
        t1 = tmp.tile([P, CN], F32, tag="t1", name="t1")
        nc.vector.tensor_tensor(t1[:], cm_ap(b), vs_sb[b][:], op=mult)
        nc.vector.tensor_tensor(vs_sb[b][:], t0[:], t1[:], op=add)
        if t < iters - 1:
            s = tmp.tile([P, CN], F32, tag="s", name="s")
            nc.vector.tensor_tensor(s[:], g_ap(b), vs_sb[b][:], op=mult)
            nc.scalar.activation(y_sb[b][:], s[:], Tanh)

    def decode(b):
        vf = tmp.tile([P, CN], F32, tag="vf", name="vf")
        nc.vector.tensor_tensor(vf[:], vs_sb[b][:], bias_ap(b), op=sub)
        ap = ps.tile([P, PSW], F32, tag="ps", name="dec")
        for c in range(CN):
            nc.tensor.matmul(
                ap[0:1, 0:ADIM],
                vf[:, c:c + 1],
                dt_sb[:, b * CN * ADIM + c * ADIM: b * CN * ADIM + (c + 1) * ADIM],
                start=(c == 0), stop=(c == CN - 1),
            )
        nc.vector.tensor_copy(act_sb[0:1, b * ADIM:(b + 1) * ADIM], ap[0:1, 0:ADIM])

    for phase in PHASES:
        for t in range(iters):
            for b in phase:
                matvec(b, t)
        for b in phase:
            decode(b)
    nc.sync.dma_start(out_ap[:], act_sb[0:1, :])


def build_nc(iters=ITERS):
    nc = bacc.Bacc(
        "TRN2", target_bir_lowering=False, debug=False, enable_asserts=False,
    )
    ins = {}
    ins["WT"] = nc.dram_tensor("WT", [BPC, P, CN * N], BF16, kind="ExternalInput").ap()
    ins["ETall"] = nc.dram_tensor("ETall", [OBS, BPC * N], BF16, kind="ExternalInput").ap()
    ins["obsT"] = nc.dram_tensor("obsT", [OBS, BPC], BF16, kind="ExternalInput").ap()
    ins["consts"] = nc.dram_tensor(
        "consts", [P, BPC * NCONST * CN], F32, kind="ExternalInput").ap()
    ins["DTall"] = nc.dram_tensor(
        "DTall", [P, BPC * CN * ADIM], F32, kind="ExternalInput").ap()
    out_ap = nc.dram_tensor("act", [BPC, ADIM], F32, kind="ExternalOutput").ap()

    with tile.TileContext(nc) as tc:
        with ExitStack() as ctx:
            pools = make_pools(ctx, tc)
            kernel_body(ctx, tc, ins, out_ap, iters, pools)
    nc.compile()
    return nc


def prep_in_maps(obs, v0, tau, gain, bias, W, mask, E, D):
    f = np.float32
    obs, v0, tau, gain, bias, W, mask, E, D = [
        np.asarray(x, dtype=f) for x in (obs, v0, tau, gain, bias, W, mask, E, D)
    ]
    am = (DT / tau) * mask                    # [64, N]
    cm = (1.0 - DT / tau) * mask
    Wf = W * am[:, :, None] * mask[:, None, :]
    WT = np.ascontiguousarray(Wf.transpose(0, 2, 1)).reshape(
        B_FULL, P, CN * N).astype(BF16_NP)
    ETp = np.ascontiguousarray(
        (E * am[:, :, None]).transpose(0, 2, 1)).astype(BF16_NP)  # [64, OBS, N]
    DTp = np.ascontiguousarray(D.transpose(0, 2, 1)).reshape(B_FULL, P, CN * ADIM)
    obsT = np.ascontiguousarray(obs.T).astype(BF16_NP)  # [OBS, 64]
    vs0 = (v0 + bias).reshape(B_FULL, P, CN)
    cmS = cm.reshape(B_FULL, P, CN)
    gS = gain.reshape(B_FULL, P, CN)
    bS = bias.reshape(B_FULL, P, CN)
    bcS = (bias * (1.0 - cm)).reshape(B_FULL, P, CN)

    in_maps = []
    for core in range(NCORES):
        s = slice(core * BPC, (core + 1) * BPC)
        # consts [128, BPC*5*8]: per b: cm, g, bias, vs0, bc
        cst = np.empty((P, BPC * NCONST * CN), f)
        for i, b in enumerate(range(core * BPC, (core + 1) * BPC)):
            for k, arr in enumerate((cmS, gS, bS, vs0, bcS)):
                cst[:, (i * NCONST + k) * CN:(i * NCONST + k + 1) * CN] = arr[b]
        et = np.ascontiguousarray(
            ETp[s].transpose(1, 0, 2).reshape(OBS, BPC * N))
        dtall = np.ascontiguousarray(
            DTp[s].transpose(1, 0, 2).reshape(P, BPC * CN * ADIM))
        in_maps.append({
            "WT": np.ascontiguousarray(WT[s]),
            "ETall": et,
            "obsT": np.ascontiguousarray(obsT[:, s]),
            "consts": cst,
            "DTall": dtall,
        })
    return in_maps


_NC_CACHE = None


def _get_nc():
    global _NC_CACHE
    if _NC_CACHE is None:
        _NC_CACHE = build_nc()
    return _NC_CACHE


def kernel(obs, v0, tau, gain, bias, W, mask, E, D):
    nc = _get_nc()
    in_maps = prep_in_maps(obs, v0, tau, gain, bias, W, mask, E, D)
    res = run_bass_kernel_spmd(nc, in_maps, core_ids=list(range(NCORES)))
    return np.concatenate([res.results[c]["act"] for c in range(NCORES)], axis=0)
